# revision 25
# baseline (speedup 1.0000x reference)
"""CRF loss (forward-algorithm log-partition + joint LLH) on 8 Trainium2 cores.

Strategy
--------
Data parallel over batch: each of the 8 cores handles 128 batch rows.

Denominator (log-partition): the 512-step forward scan runs in *scaled
probability space* so each step is a small PE matmul followed by one DVE
tensor-tensor multiply:

    gamma_{s+1} = (c*A)^T gamma_s  (*) E_{s+1},   E_s = exp(emissions_s)

A = exp(Ttt) is the 48x48 transition kernel. The overflow-control constant c
is folded into the matmul weights; the final log gets +511*ln(1/c) added back
on the host. The scan runs forward from s=0 and backward from s=511
simultaneously (meet in the middle), stacked in one [96, 64] tile: partitions
0-47 hold the forward chain, 48-95 the backward chain, with
blockdiag(cA, (cA)^T) weights. Two such fused chains (batch columns 0-63 and
64-127) interleave to hide cross-engine latency. Emissions stream in bf16.

Numerator (joint LLH): no gathers. Host builds (a) a bf16 one-hot of the tag
path over the flat [S*T] emission axis and (b) an exact f32 count matrix C_b
over the 50x50 transition-pair table (START/END pairs included). On device the
numerator is five fused multiply-accumulate instructions on the otherwise-idle
GpSimd engine, fully overlapped with the scan:

    num_b = sum(em_flat * onehot) + sum(trans_flat * C_b)

Host does only: sharding, layout transforms, one-hot/count construction from
int tags, tiny final mean over the 1024 per-b partials and the +511*ln(1/c)
constant.
"""

import numpy as np

B, S, T = 1024, 512, 48
TT2 = (T + 2) * (T + 2)     # 2500-entry padded transition table
NCORES = 8
BL = B // NCORES            # 128 batch rows per core
NG = 2                      # fused chains per core (64 batch cols each)
W = 64                      # batch columns per chain
HT = 256                    # tick 0 = init, ticks 1..255 = scan, meet after
GK = 8                      # ticks per emission super-tile (DMA/exp batch)
NCH = 8                     # numerator emission chunks

_CACHE = {}
_TRACE = False
LAST = {"exec_ns": None, "results": None, "trace": None}


def _build_module():
    from concourse import bacc
    import concourse.bass as bass
    import concourse.mybir as mybir
    import concourse.tile as tile

    f32 = mybir.dt.float32
    bf16 = mybir.dt.bfloat16

    nc = bacc.Bacc(
        "TRN2",
        target_bir_lowering=False,
        debug=False,
        enable_asserts=False,
    )

    emi = nc.dram_tensor("emi", [NG, HT, 96, W], bf16, kind="ExternalInput").ap()
    emf = nc.dram_tensor("emf", [BL, S * T], bf16, kind="ExternalInput").ap()
    selv = nc.dram_tensor("selv", [BL, S * T], bf16, kind="ExternalInput").ap()
    cmat = nc.dram_tensor("cmat", [BL, TT2], bf16, kind="ExternalInput").ap()
    trn = nc.dram_tensor("trans", [TT2], bf16, kind="ExternalInput").ap()
    wmat = nc.dram_tensor("wmat", [96, 96], bf16, kind="ExternalInput").ap()
    initb = nc.dram_tensor("initb", [96, 1], f32, kind="ExternalInput").ap()
    den = nc.dram_tensor("den", [1, NG * W], f32, kind="ExternalOutput").ap()
    num = nc.dram_tensor("num", [BL, 1], f32, kind="ExternalOutput").ap()

    AF = mybir.ActivationFunctionType
    OP = mybir.AluOpType

    CH = S * T // NCH  # 6144 flat elems per chunk

    with tile.TileContext(nc) as tc:
        with (
            tc.tile_pool(name="const", bufs=1) as const,
            tc.tile_pool(name="raw", bufs=4) as rawp,
            tc.tile_pool(name="ex", bufs=4) as exp_,
            tc.tile_pool(name="gam", bufs=3) as gam,
            tc.tile_pool(name="nmr", bufs=1) as nmr,
            tc.tile_pool(name="emp", bufs=3) as emp,
            tc.tile_pool(name="slp", bufs=3) as slp,
            tc.tile_pool(name="fin", bufs=1) as fin,
            tc.tile_pool(name="ps", bufs=2, space="PSUM") as psp,
            tc.tile_pool(name="psfin", bufs=1, space="PSUM") as psf,
        ):
            # ---- constants ----
            w_sb = const.tile([96, 96], bf16, tag="w")
            ib_sb = const.tile([96, 1], f32, tag="ib")
            ones48 = const.tile([48, 1], f32, tag="ones")
            nc.vector.memset(ones48[:], 1.0)

            # ---- scan ----
            emi_t = emi.rearrange("g t p w -> g p t w")
            ngrp = HT // GK

            raw = [[None] * ngrp for _ in range(NG)]
            ex = [[None] * ngrp for _ in range(NG)]

            def load_group(g, grp):
                r = rawp.tile([96, GK, W], bf16, tag=f"raw{g}")
                nc.sync.dma_start(
                    out=r[:], in_=emi_t[g, :, grp * GK : (grp + 1) * GK, :]
                )
                e = exp_.tile([96, GK, W], f32, tag=f"ex{g}")
                nc.scalar.activation(
                    e[:].rearrange("p a b -> p (a b)"),
                    r[:].rearrange("p a b -> p (a b)"),
                    AF.Exp,
                )
                raw[g][grp] = r
                ex[g][grp] = e

            # first-tile DMAs go out first (exp-init needs them + ib);
            # w only gates the first matmul
            for g in range(NG):
                load_group(g, 0)
            nc.sync.dma_start(out=ib_sb[:], in_=initb)
            nc.sync.dma_start(out=w_sb[:], in_=wmat)
            # tiny Sync-queue marker right after the scan-critical DMA
            # prefix: gates all numerator HBM traffic (single contiguous
            # descriptor, ~0.5us)
            gate = const.tile([1, W], bf16, tag="gate")
            nc.sync.dma_start(
                out=gate[:],
                in_=bass.AP(tensor=emi.tensor, offset=0, ap=[[0, 1], [1, W]]),
            )
            gcur = [None, None]
            for g in range(NG):
                # init: fwd row-block = exp(em_0 + trans[START,:]),
                #       bwd row-block = exp(em_511 + trans[:,END])
                g0 = gam.tile([96, W], bf16, tag=f"g{g}")
                nc.scalar.activation(
                    g0[:], raw[g][0][:, 0, :], AF.Exp, bias=ib_sb[:], scale=1.0
                )
                gcur[g] = g0

            for t in range(1, HT):
                grp, slot = divmod(t, GK)
                for g in range(NG):
                    if raw[g][grp] is None:
                        load_group(g, grp)
                    ps = psp.tile([96, W], f32, tag=f"ps{g}")
                    mm = nc.tensor.matmul(
                        ps[:], w_sb[:], gcur[g][:], start=True, stop=True
                    )
                    if t > 1:
                        # weights stay resident in the PE array; only the
                        # first matmul of each chain self-loads them.
                        mm.ins.ldweights = False
                    gn = gam.tile([96, W], bf16, tag=f"g{g}")
                    nc.vector.tensor_mul(
                        out=gn[:], in0=ps[:], in1=ex[g][grp][:, slot, :]
                    )
                    gcur[g] = gn

            # ---- meet in the middle:  Z*c^511 = (cA^T f_255)^T ghat_256 ----
            zt = fin.tile([48, NG * W], f32, tag="zt")
            for g in range(NG):
                psm = psf.tile([96, W], f32, tag=f"meet{g}")
                mm = nc.tensor.matmul(
                    psm[:], w_sb[:], gcur[g][:], start=True, stop=True
                )
                mm.ins.ldweights = False
                gmv = fin.tile([48, W], bf16, tag=f"gmv{g}")
                nc.sync.dma_start(out=gmv[:], in_=gcur[g][48:96, :])
                nc.vector.tensor_mul(
                    out=zt[:, g * W : (g + 1) * W], in0=psm[0:48, :], in1=gmv[:]
                )
            psz = psf.tile([1, NG * W], f32, tag="psz")
            nc.tensor.matmul(psz[:], ones48[:], zt[:], start=True, stop=True)
            dsb = fin.tile([1, NG * W], f32, tag="dsb")
            nc.scalar.activation(dsb[:], psz[:], AF.Ln)
            nc.sync.dma_start(out=den, in_=dsb[:])

            # ---- numerator: GpSimd only, fully overlapped with the scan ----
            # Scheduler fence: nothing below may be reordered before the scan
            # (without it, the final DVE reduces get scheduled mid-scan and
            # block the Vector stream on GpSimd progress for ~25us).
            tc.no_sync_barrier()
            # All numerator HBM traffic is gated on the scan's first tile
            # groups (via WAR deps on the gate marker) and self-paced by the
            # bufs=2 chunk pools, so it cannot starve the scan's tile loads.
            tab = nmr.tile([BL, TT2], bf16, tag="tab")
            cc = nmr.tile([BL, TT2], bf16, tag="cc")
            ct = nmr.tile([BL, TT2], f32, tag="ct")

            # chunk products em*onehot in exact bf16, summed pairwise into
            # f32 (every add below is exact: products are bf16-representable)
            pb = [
                nmr.tile([BL, CH], bf16, name="pb0", tag="pb0"),
                nmr.tile([BL, CH], bf16, name="pb1", tag="pb1"),
            ]
            acc = nmr.tile([BL, CH], f32, tag="nacc")
            scr = nmr.tile([BL, CH], f32, tag="nscr")

            # free-axis add-ladders on GpSimd (down to `stop` elems)
            def ladder(tile_ap, n, stop):
                while n > stop:
                    h = n // 2
                    nc.gpsimd.tensor_tensor(
                        out=tile_ap[:, 0:h],
                        in0=tile_ap[:, 0:h],
                        in1=tile_ap[:, h : 2 * h],
                        op=OP.add,
                    )
                    if n % 2:
                        nc.gpsimd.tensor_tensor(
                            out=tile_ap[:, 0:1],
                            in0=tile_ap[:, 0:1],
                            in1=tile_ap[:, n - 1 : n],
                            op=OP.add,
                        )
                    n = h
                return n

            ntr = None
            for q in range(NCH):
                et = emp.tile([BL, CH], bf16, tag="em")
                st = slp.tile([BL, CH], bf16, tag="sl")
                if q < 3:
                    # WAR dep: chunk DMA waits for the scan-start marker
                    nc.gpsimd.tensor_copy(out=et[0:1, 0:W], in_=gate[:])
                    nc.gpsimd.tensor_copy(out=st[0:1, 0:W], in_=gate[:])
                nc.gpsimd.dma_start(
                    out=et[:], in_=emf[:, q * CH : (q + 1) * CH]
                )
                nc.gpsimd.dma_start(
                    out=st[:], in_=selv[:, q * CH : (q + 1) * CH]
                )
                nc.gpsimd.tensor_tensor(
                    out=pb[q % 2][:], in0=et[:], in1=st[:], op=OP.mult
                )
                if q % 2:
                    dst = acc if q == 1 else scr
                    nc.gpsimd.tensor_tensor(
                        out=dst[:], in0=pb[0][:], in1=pb[1][:], op=OP.add
                    )
                    if q > 1:
                        nc.gpsimd.tensor_tensor(
                            out=acc[:], in0=acc[:], in1=scr[:], op=OP.add
                        )
                if q == 2:
                    # trans-table traffic fires once the startup burst is over
                    nc.gpsimd.dma_start(
                        out=tab[:],
                        in_=bass.AP(
                            tensor=trn.tensor, offset=0, ap=[[0, BL], [1, TT2]]
                        ),
                    )
                    nc.gpsimd.dma_start(out=cc[:], in_=cmat)
                if q == 4:
                    # trans contraction mid-stream so it's off the tail
                    nc.gpsimd.tensor_tensor(
                        out=ct[:], in0=cc[:], in1=tab[:], op=OP.mult
                    )
                    ntr = ladder(ct, TT2, 96)

            nem = ladder(acc, CH, 96)

            # tiny post-scan finish on DVE + output
            nsum = nmr.tile([BL, 1], f32, tag="nsum")
            tsum = nmr.tile([BL, 1], f32, tag="tsum")
            nc.vector.tensor_reduce(
                nsum[:], acc[:, 0:nem], axis=mybir.AxisListType.X, op=OP.add
            )
            nc.vector.tensor_reduce(
                tsum[:], ct[:, 0:ntr], axis=mybir.AxisListType.X, op=OP.add
            )
            nc.vector.tensor_add(out=nsum[:], in0=nsum[:], in1=tsum[:])
            nc.gpsimd.dma_start(out=num, in_=nsum[:])

    nc.compile()
    return nc


def _prep(emissions, tags, transitions):
    import ml_dtypes

    em = np.ascontiguousarray(emissions, dtype=np.float32)
    tg = np.ascontiguousarray(tags).astype(np.int64)
    tr = np.ascontiguousarray(transitions, dtype=np.float32)

    A = np.exp(tr[:T, :T].astype(np.float64))
    c = 1.0 / (A.sum(axis=0).mean() * np.exp(0.5))
    logc = float(np.log(c))
    cA = c * A
    w1 = np.zeros((96, 96), np.float64)
    w1[:48, :48] = cA
    w1[48:, 48:] = cA.T
    wmat = np.ascontiguousarray(w1, dtype=ml_dtypes.bfloat16)
    initb = np.ascontiguousarray(
        np.concatenate([tr[T, :T], tr[:T, T + 1]])[:, None], dtype=np.float32
    )

    # flat bf16 emissions + one-hot of the tag path over the [S*T] axis
    emf = em.reshape(B, S * T).astype(ml_dtypes.bfloat16)
    s_idx = np.arange(S)[None, :]
    em_idx = (s_idx * T + tg).astype(np.int64)               # [B, S]
    selv = np.zeros((B, S * T), ml_dtypes.bfloat16)
    selv[np.arange(B)[:, None], em_idx] = 1.0

    # exact transition-pair counts over the padded 50x50 table
    pair_mid = tg[:, :-1] * (T + 2) + tg[:, 1:]              # [B, S-1]
    pair_start = (T * (T + 2) + tg[:, 0])[:, None]           # START -> tag0
    pair_end = (tg[:, -1] * (T + 2) + T + 1)[:, None]        # taglast -> END
    pairs = np.concatenate([pair_mid, pair_start, pair_end], axis=1)
    boff = (np.arange(B) * TT2)[:, None]
    cmat = (
        np.bincount((pairs + boff).ravel(), minlength=B * TT2)
        .reshape(B, TT2)
        .astype(ml_dtypes.bfloat16)
    )
    trf = np.ascontiguousarray(tr.reshape(-1).astype(ml_dtypes.bfloat16))

    # emi[g, t, p, w]: p<48 -> em[b, s=t, j=p]; p>=48 -> em[b, s=511-t, j=p-48]
    # with b = core*128 + g*64 + w
    in_maps = []
    for core in range(NCORES):
        b0 = core * BL
        em_c = em[b0 : b0 + BL]                          # [128, 512, 48] view
        x = em_c.transpose(1, 2, 0)                      # [512, 48, 128] view
        tops = x[0:HT]                                   # [256, 48, 128]
        bots = x[S - 1 : HT - 1 : -1]                    # s = 511..256
        emi = np.empty((NG, HT, 96, W), ml_dtypes.bfloat16)
        for g in range(NG):
            emi[g, :, :48, :] = tops[:, :, g * W : (g + 1) * W]
            emi[g, :, 48:, :] = bots[:, :, g * W : (g + 1) * W]
        in_maps.append(
            {
                "emi": emi,
                "emf": np.ascontiguousarray(emf[b0 : b0 + BL]),
                "selv": np.ascontiguousarray(selv[b0 : b0 + BL]),
                "cmat": np.ascontiguousarray(cmat[b0 : b0 + BL]),
                "trans": trf,
                "wmat": wmat,
                "initb": initb,
            }
        )
    return in_maps, logc


def kernel(emissions, tags, transitions):
    from concourse.bass_utils import run_bass_kernel_spmd

    if "nc" not in _CACHE:
        _CACHE["nc"] = _build_module()
    nc = _CACHE["nc"]

    in_maps, logc = _prep(emissions, tags, transitions)
    res = run_bass_kernel_spmd(
        nc, in_maps, core_ids=list(range(NCORES)), trace=_TRACE
    )
    LAST["exec_ns"] = res.exec_time_ns
    LAST["results"] = res.results
    LAST["trace"] = res.instructions_and_trace

    total = 0.0
    for core in range(NCORES):
        r = res.results[core]
        d = r["den"].reshape(-1).astype(np.float64)     # ln(c^511 * Z_b)
        n = r["num"].reshape(-1).astype(np.float64)
        total += np.sum(n - (d - 511.0 * logc))
    return np.asarray(total / B, dtype=np.float32)


# revision 26
# speedup vs baseline: 1.1289x; 1.1289x over previous
"""CRF loss (forward-algorithm log-partition + joint LLH) on 8 Trainium2 cores.

Strategy
--------
Data parallel over batch: each of the 8 cores handles 128 batch rows.

Denominator (log-partition): the 512-step forward scan runs in *scaled
probability space* so each step is a small PE matmul followed by one DVE
tensor-tensor multiply:

    gamma_{s+1} = (c*A)^T gamma_s  (*) E_{s+1},   E_s = exp(emissions_s)

A = exp(Ttt) is the 48x48 transition kernel. The overflow-control constant c
is folded into the matmul weights; the final log gets +511*ln(1/c) added back
on the host. The scan runs forward from s=0 and backward from s=511
simultaneously (meet in the middle), stacked in one [96, 64] tile: partitions
0-47 hold the forward chain, 48-95 the backward chain, with
blockdiag(cA, (cA)^T) weights. Two such fused chains (batch columns 0-63 and
64-127) interleave to hide cross-engine latency. Emissions stream in bf16.

Numerator (joint LLH): no gathers. Host builds (a) a bf16 one-hot of the tag
path over the flat [S*T] emission axis and (b) an exact f32 count matrix C_b
over the 50x50 transition-pair table (START/END pairs included). On device the
numerator is five fused multiply-accumulate instructions on the otherwise-idle
GpSimd engine, fully overlapped with the scan:

    num_b = sum(em_flat * onehot) + sum(trans_flat * C_b)

Host does only: sharding, layout transforms, one-hot/count construction from
int tags, tiny final mean over the 1024 per-b partials and the +511*ln(1/c)
constant.
"""

import numpy as np

B, S, T = 1024, 512, 48
TT2 = (T + 2) * (T + 2)     # 2500-entry padded transition table
NCORES = 8
BL = B // NCORES            # 128 batch rows per core
NG = 2                      # fused chains per core (64 batch cols each)
W = 64                      # batch columns per chain
HT = 256                    # tick 0 = init, ticks 1..255 = scan, meet after
GK = 8                      # ticks per emission super-tile (DMA/exp batch)
NCH = 8                     # numerator emission chunks

_CACHE = {}
_TRACE = False
LAST = {"exec_ns": None, "results": None, "trace": None}


def _build_module():
    from concourse import bacc
    import concourse.bass as bass
    import concourse.mybir as mybir
    import concourse.tile as tile

    f32 = mybir.dt.float32
    bf16 = mybir.dt.bfloat16

    nc = bacc.Bacc(
        "TRN2",
        target_bir_lowering=False,
        debug=False,
        enable_asserts=False,
    )

    emi = nc.dram_tensor("emi", [NG, HT, 96, W], bf16, kind="ExternalInput").ap()
    emf = nc.dram_tensor("emf", [BL, S * T], bf16, kind="ExternalInput").ap()
    selv = nc.dram_tensor("selv", [BL, S * T], mybir.dt.int8, kind="ExternalInput").ap()
    cmat = nc.dram_tensor("cmat", [BL, TT2], bf16, kind="ExternalInput").ap()
    trn = nc.dram_tensor("trans", [TT2], bf16, kind="ExternalInput").ap()
    wmat = nc.dram_tensor("wmat", [96, 96], bf16, kind="ExternalInput").ap()
    initb = nc.dram_tensor("initb", [96, 1], f32, kind="ExternalInput").ap()
    den = nc.dram_tensor("den", [1, NG * W], f32, kind="ExternalOutput").ap()
    num = nc.dram_tensor("num", [BL, 1], f32, kind="ExternalOutput").ap()

    AF = mybir.ActivationFunctionType
    OP = mybir.AluOpType

    CH = S * T // NCH  # 6144 flat elems per chunk

    with tile.TileContext(nc) as tc:
        with (
            tc.tile_pool(name="const", bufs=1) as const,
            tc.tile_pool(name="raw", bufs=4) as rawp,
            tc.tile_pool(name="ex", bufs=4) as exp_,
            tc.tile_pool(name="gam", bufs=3) as gam,
            tc.tile_pool(name="nmr", bufs=1) as nmr,
            tc.tile_pool(name="emp", bufs=2) as emp,
            tc.tile_pool(name="slp", bufs=2) as slp,
            tc.tile_pool(name="fin", bufs=1) as fin,
            tc.tile_pool(name="ps", bufs=2, space="PSUM") as psp,
            tc.tile_pool(name="psfin", bufs=1, space="PSUM") as psf,
        ):
            # ---- constants ----
            w_sb = const.tile([96, 96], bf16, tag="w")
            ib_sb = const.tile([96, 1], f32, tag="ib")
            ones48 = const.tile([48, 1], f32, tag="ones")
            nc.vector.memset(ones48[:], 1.0)

            # ---- scan ----
            emi_t = emi.rearrange("g t p w -> g p t w")
            ngrp = HT // GK

            raw = [[None] * ngrp for _ in range(NG)]
            ex = [[None] * ngrp for _ in range(NG)]

            def load_group(g, grp):
                r = rawp.tile([96, GK, W], bf16, tag=f"raw{g}")
                nc.sync.dma_start(
                    out=r[:], in_=emi_t[g, :, grp * GK : (grp + 1) * GK, :]
                )
                e = exp_.tile([96, GK, W], f32, tag=f"ex{g}")
                nc.scalar.activation(
                    e[:].rearrange("p a b -> p (a b)"),
                    r[:].rearrange("p a b -> p (a b)"),
                    AF.Exp,
                )
                raw[g][grp] = r
                ex[g][grp] = e

            # first-tile DMAs go out first (exp-init needs them + ib);
            # w only gates the first matmul
            for g in range(NG):
                load_group(g, 0)
            nc.sync.dma_start(out=ib_sb[:], in_=initb)
            nc.sync.dma_start(out=w_sb[:], in_=wmat)
            # tiny Sync-queue marker right after the scan-critical DMA
            # prefix: gates all numerator HBM traffic (single contiguous
            # descriptor, ~0.5us)
            gate = const.tile([1, W], bf16, tag="gate")
            nc.sync.dma_start(
                out=gate[:],
                in_=bass.AP(tensor=emi.tensor, offset=0, ap=[[0, 1], [1, W]]),
            )
            gcur = [None, None]
            for g in range(NG):
                # init: fwd row-block = exp(em_0 + trans[START,:]),
                #       bwd row-block = exp(em_511 + trans[:,END])
                g0 = gam.tile([96, W], bf16, tag=f"g{g}")
                nc.scalar.activation(
                    g0[:], raw[g][0][:, 0, :], AF.Exp, bias=ib_sb[:], scale=1.0
                )
                gcur[g] = g0

            for t in range(1, HT):
                grp, slot = divmod(t, GK)
                for g in range(NG):
                    if raw[g][grp] is None:
                        load_group(g, grp)
                    ps = psp.tile([96, W], f32, tag=f"ps{g}")
                    mm = nc.tensor.matmul(
                        ps[:], w_sb[:], gcur[g][:], start=True, stop=True
                    )
                    if t > 1:
                        # weights stay resident in the PE array; only the
                        # first matmul of each chain self-loads them.
                        mm.ins.ldweights = False
                    gn = gam.tile([96, W], bf16, tag=f"g{g}")
                    nc.vector.tensor_mul(
                        out=gn[:], in0=ps[:], in1=ex[g][grp][:, slot, :]
                    )
                    gcur[g] = gn

            # ---- meet in the middle:  Z*c^511 = (cA^T f_255)^T ghat_256 ----
            zt = fin.tile([48, NG * W], f32, tag="zt")
            for g in range(NG):
                psm = psf.tile([96, W], f32, tag=f"meet{g}")
                mm = nc.tensor.matmul(
                    psm[:], w_sb[:], gcur[g][:], start=True, stop=True
                )
                mm.ins.ldweights = False
                gmv = fin.tile([48, W], bf16, tag=f"gmv{g}")
                nc.sync.dma_start(out=gmv[:], in_=gcur[g][48:96, :])
                nc.vector.tensor_mul(
                    out=zt[:, g * W : (g + 1) * W], in0=psm[0:48, :], in1=gmv[:]
                )
            psz = psf.tile([1, NG * W], f32, tag="psz")
            nc.tensor.matmul(psz[:], ones48[:], zt[:], start=True, stop=True)
            dsb = fin.tile([1, NG * W], f32, tag="dsb")
            nc.scalar.activation(dsb[:], psz[:], AF.Ln)
            nc.sync.dma_start(out=den, in_=dsb[:])

            # ---- numerator: GpSimd only, fully overlapped with the scan ----
            # Scheduler fence: nothing below may be reordered before the scan
            # (without it, the final DVE reduces get scheduled mid-scan and
            # block the Vector stream on GpSimd progress for ~25us).
            tc.no_sync_barrier()
            # All numerator HBM traffic is gated on the scan's first tile
            # groups (via WAR deps on the gate marker) and self-paced by the
            # bufs=2 chunk pools, so it cannot starve the scan's tile loads.
            tab = nmr.tile([BL, TT2], bf16, tag="tab")
            cc = nmr.tile([BL, TT2], bf16, tag="cc")
            ct = nmr.tile([BL, TT2], f32, tag="ct")

            # chunk products em*onehot in exact bf16, summed pairwise into
            # f32 (every add below is exact: products are bf16-representable)
            pb = [
                nmr.tile([BL, CH], bf16, name="pb0", tag="pb0"),
                nmr.tile([BL, CH], bf16, name="pb1", tag="pb1"),
            ]
            acc = nmr.tile([BL, CH], f32, tag="nacc")
            scr = nmr.tile([BL, CH], f32, tag="nscr")

            # free-axis add-ladders on GpSimd (down to `stop` elems)
            def ladder(tile_ap, n, stop):
                while n > stop:
                    h = n // 2
                    nc.gpsimd.tensor_tensor(
                        out=tile_ap[:, 0:h],
                        in0=tile_ap[:, 0:h],
                        in1=tile_ap[:, h : 2 * h],
                        op=OP.add,
                    )
                    if n % 2:
                        nc.gpsimd.tensor_tensor(
                            out=tile_ap[:, 0:1],
                            in0=tile_ap[:, 0:1],
                            in1=tile_ap[:, n - 1 : n],
                            op=OP.add,
                        )
                    n = h
                return n

            ntr = None
            for q in range(NCH):
                et = emp.tile([BL, CH], bf16, tag="em")
                st = slp.tile([BL, CH], bf16, tag="sl")
                if q < 2:
                    # WAR dep: chunk DMA waits for the scan-start marker
                    nc.gpsimd.tensor_copy(out=et[0:1, 0:W], in_=gate[:])
                    nc.gpsimd.tensor_copy(out=st[0:1, 0:W], in_=gate[:])
                nc.gpsimd.dma_start(
                    out=et[:], in_=emf[:, q * CH : (q + 1) * CH]
                )
                nc.gpsimd.dma_start(
                    out=st[:], in_=selv[:, q * CH : (q + 1) * CH]
                )
                nc.gpsimd.tensor_tensor(
                    out=pb[q % 2][:], in0=et[:], in1=st[:], op=OP.mult
                )
                if q % 2:
                    dst = acc if q == 1 else scr
                    nc.gpsimd.tensor_tensor(
                        out=dst[:], in0=pb[0][:], in1=pb[1][:], op=OP.add
                    )
                    if q > 1:
                        nc.gpsimd.tensor_tensor(
                            out=acc[:], in0=acc[:], in1=scr[:], op=OP.add
                        )
                if q == 2:
                    # trans-table traffic fires once the startup burst is over
                    nc.gpsimd.dma_start(
                        out=tab[:],
                        in_=bass.AP(
                            tensor=trn.tensor, offset=0, ap=[[0, BL], [1, TT2]]
                        ),
                    )
                    nc.gpsimd.dma_start(out=cc[:], in_=cmat)

            nc.gpsimd.tensor_tensor(out=ct[:], in0=cc[:], in1=tab[:], op=OP.mult)
            ntr = ladder(ct, TT2, 96)
            nem = ladder(acc, CH, 96)

            # tiny post-scan finish on DVE + output
            nsum = nmr.tile([BL, 1], f32, tag="nsum")
            tsum = nmr.tile([BL, 1], f32, tag="tsum")
            nc.vector.tensor_reduce(
                nsum[:], acc[:, 0:nem], axis=mybir.AxisListType.X, op=OP.add
            )
            nc.vector.tensor_reduce(
                tsum[:], ct[:, 0:ntr], axis=mybir.AxisListType.X, op=OP.add
            )
            nc.vector.tensor_add(out=nsum[:], in0=nsum[:], in1=tsum[:])
            nc.gpsimd.dma_start(out=num, in_=nsum[:])

    nc.compile()
    return nc


def _prep(emissions, tags, transitions):
    import ml_dtypes

    em = np.ascontiguousarray(emissions, dtype=np.float32)
    tg = np.ascontiguousarray(tags).astype(np.int64)
    tr = np.ascontiguousarray(transitions, dtype=np.float32)

    A = np.exp(tr[:T, :T].astype(np.float64))
    c = 1.0 / (A.sum(axis=0).mean() * np.exp(0.5))
    logc = float(np.log(c))
    cA = c * A
    w1 = np.zeros((96, 96), np.float64)
    w1[:48, :48] = cA
    w1[48:, 48:] = cA.T
    wmat = np.ascontiguousarray(w1, dtype=ml_dtypes.bfloat16)
    initb = np.ascontiguousarray(
        np.concatenate([tr[T, :T], tr[:T, T + 1]])[:, None], dtype=np.float32
    )

    # flat bf16 emissions + one-hot of the tag path over the [S*T] axis
    emf = em.reshape(B, S * T).astype(ml_dtypes.bfloat16)
    s_idx = np.arange(S)[None, :]
    em_idx = (s_idx * T + tg).astype(np.int64)               # [B, S]
    selv = np.zeros((B, S * T), np.int8)
    selv[np.arange(B)[:, None], em_idx] = 1

    # exact transition-pair counts over the padded 50x50 table
    pair_mid = tg[:, :-1] * (T + 2) + tg[:, 1:]              # [B, S-1]
    pair_start = (T * (T + 2) + tg[:, 0])[:, None]           # START -> tag0
    pair_end = (tg[:, -1] * (T + 2) + T + 1)[:, None]        # taglast -> END
    pairs = np.concatenate([pair_mid, pair_start, pair_end], axis=1)
    boff = (np.arange(B) * TT2)[:, None]
    cmat = (
        np.bincount((pairs + boff).ravel(), minlength=B * TT2)
        .reshape(B, TT2)
        .astype(ml_dtypes.bfloat16)
    )
    trf = np.ascontiguousarray(tr.reshape(-1).astype(ml_dtypes.bfloat16))

    # emi[g, t, p, w]: p<48 -> em[b, s=t, j=p]; p>=48 -> em[b, s=511-t, j=p-48]
    # with b = core*128 + g*64 + w
    in_maps = []
    for core in range(NCORES):
        b0 = core * BL
        em_c = em[b0 : b0 + BL]                          # [128, 512, 48] view
        x = em_c.transpose(1, 2, 0)                      # [512, 48, 128] view
        tops = x[0:HT]                                   # [256, 48, 128]
        bots = x[S - 1 : HT - 1 : -1]                    # s = 511..256
        emi = np.empty((NG, HT, 96, W), ml_dtypes.bfloat16)
        for g in range(NG):
            emi[g, :, :48, :] = tops[:, :, g * W : (g + 1) * W]
            emi[g, :, 48:, :] = bots[:, :, g * W : (g + 1) * W]
        in_maps.append(
            {
                "emi": emi,
                "emf": np.ascontiguousarray(emf[b0 : b0 + BL]),
                "selv": np.ascontiguousarray(selv[b0 : b0 + BL]),
                "cmat": np.ascontiguousarray(cmat[b0 : b0 + BL]),
                "trans": trf,
                "wmat": wmat,
                "initb": initb,
            }
        )
    return in_maps, logc


def kernel(emissions, tags, transitions):
    from concourse.bass_utils import run_bass_kernel_spmd

    if "nc" not in _CACHE:
        _CACHE["nc"] = _build_module()
    nc = _CACHE["nc"]

    in_maps, logc = _prep(emissions, tags, transitions)
    res = run_bass_kernel_spmd(
        nc, in_maps, core_ids=list(range(NCORES)), trace=_TRACE
    )
    LAST["exec_ns"] = res.exec_time_ns
    LAST["results"] = res.results
    LAST["trace"] = res.instructions_and_trace

    total = 0.0
    for core in range(NCORES):
        r = res.results[core]
        d = r["den"].reshape(-1).astype(np.float64)     # ln(c^511 * Z_b)
        n = r["num"].reshape(-1).astype(np.float64)
        total += np.sum(n - (d - 511.0 * logc))
    return np.asarray(total / B, dtype=np.float32)
